# revision 2
# baseline (speedup 1.0000x reference)
"""GCE-GNN session-rec forward for Trainium2.

Phase 1 (host, numpy): per-session graph construction + tiny GRU-style GNN
  (B=256 sessions, L=50, D=128 — ~0.5 GFLOP of irregular gather/scatter math).
Phase 2 (device, bass/tile, 8 NeuronCores): logits = reps @ emb.T
  vocab-sharded: each core reads a [128, VS] slice of emb.T and writes a
  [256, VS] slice of the output. This is the memory-bound bulk of the op
  (256 MB emb read + 512 MB logits write across the 8 cores).
"""

import numpy as np

V = 500000
L = 50
D = 128
B = 256
VTOT = V + 1

NCORES = 8
CHUNK = 512            # matmul moving-operand width (fp32 max)
EB_COLS = 2048         # emb.T columns per DMA (1 MiB per transfer)
VS = 31 * EB_COLS      # 63488 vocab columns per core
VP = VS * NCORES       # 507904 padded vocab


# ---------------------------------------------------------------------------
# Phase 1: host-side session GNN (numpy, float64 accumulation)
# ---------------------------------------------------------------------------

def _sigmoid(x):
    return 1.0 / (1.0 + np.exp(-x))


def _host_reps(seq, emb, W_in, W_out, Wz, bz, Uz, Wr, br, Ur, Wh, bh, Uh,
               Wg, bg, Wgate, bgate, Wproj, bproj):
    f = np.float64
    seq = np.asarray(seq)
    Bc, Lc = seq.shape
    BIG = emb.shape[0]  # sentinel > any valid item id

    valid = seq > 0
    lengths = valid.sum(1)

    # torch.unique(return_inverse) emulation, padded to L nodes
    sv = np.sort(np.where(valid, seq, BIG), axis=1)
    vs = sv < BIG
    is_new = vs & np.concatenate(
        [np.ones((Bc, 1), bool), sv[:, 1:] != sv[:, :-1]], axis=1)
    rank = np.cumsum(is_new, axis=1) - 1
    n_nodes = is_new.sum(1)
    buf = np.zeros((Bc, Lc + 1), sv.dtype)
    idx = np.where(is_new, rank, Lc)
    np.put_along_axis(buf, idx, sv, axis=1)
    uniq = buf[:, :Lc]
    usearch = np.where(np.arange(Lc)[None, :] < n_nodes[:, None], uniq, BIG)
    inv = np.empty((Bc, Lc), np.int64)
    for b in range(Bc):
        inv[b] = np.searchsorted(usearch[b], seq[b])
    inv = np.clip(inv, 0, Lc - 1)

    # local adjacency (binary), row-normalized
    pair_ok = valid[:, :-1] & valid[:, 1:]
    srcn = np.where(pair_ok, inv[:, :-1], 0)
    dstn = np.where(pair_ok, inv[:, 1:], 0)
    val = pair_ok.astype(f)
    multi = (n_nodes > 1).astype(f)[:, None, None]
    bidx = np.broadcast_to(np.arange(Bc)[:, None], srcn.shape)
    A_in = np.zeros((Bc, Lc, Lc), f)
    A_out = np.zeros((Bc, Lc, Lc), f)
    np.maximum.at(A_in, (bidx, dstn, srcn), val)
    np.maximum.at(A_out, (bidx, srcn, dstn), val)
    A_in *= multi
    A_out *= multi
    A_in /= (A_in.sum(2, keepdims=True) + 1e-8)
    A_out /= (A_out.sum(2, keepdims=True) + 1e-8)

    h = emb.astype(f)[uniq]  # [B, L, D]

    W_in, W_out, Wz, Uz, Wr, Ur, Wh, Uh, Wg, Wgate, Wproj = (
        a.astype(f) for a in (W_in, W_out, Wz, Uz, Wr, Ur, Wh, Uh, Wg, Wgate, Wproj))
    bz, br, bh, bg, bgate, bproj = (
        a.astype(f) for a in (bz, br, bh, bg, bgate, bproj))

    # local GRU-style GNN, one step
    m = A_in @ (h @ W_in) + A_out @ (h @ W_out)
    z = _sigmoid(m @ Wz + bz + h @ Uz)
    r = _sigmoid(m @ Wr + br + h @ Ur)
    ht = np.tanh(m @ Wh + bh + (r * h) @ Uh)
    h_local = (1.0 - z) * h + z * ht

    # global episode GNN, one step
    nvmask = (np.arange(Lc)[None, :] < n_nodes[:, None]).astype(f)
    Ag = nvmask[:, :, None] * nvmask[:, None, :] * \
        (1.0 - np.eye(Lc, dtype=f))[None]
    Ag /= (Ag.sum(2, keepdims=True) + 1e-8)
    h_global = np.where((n_nodes > 1)[:, None, None], Ag @ (h @ Wg + bg), h)

    # gather back to sequence, gate, attention pooling
    hl = np.take_along_axis(h_local, inv[:, :, None], axis=1)
    hg = np.take_along_axis(h_global, inv[:, :, None], axis=1)
    gate = _sigmoid(np.concatenate([hl, hg], axis=-1) @ Wgate + bgate)
    h_seq = gate * hl + (1.0 - gate) * hg
    last_idx = np.clip(lengths - 1, 0, Lc - 1)
    last_h = h_seq[np.arange(Bc), last_idx]
    att = np.where(valid, np.einsum('bld,bd->bl', h_seq, last_h), -1e9)
    att = att - att.max(1, keepdims=True)
    e = np.exp(att)
    alpha = e / e.sum(1, keepdims=True)
    s_g = np.einsum('bl,bld->bd', alpha, h_seq)
    reps = np.concatenate([s_g, last_h], axis=-1) @ Wproj + bproj
    return reps.astype(np.float32)  # [B, D]


# ---------------------------------------------------------------------------
# Phase 2: device kernel (built once, cached)
# ---------------------------------------------------------------------------

_NC = None


def _build_nc():
    import concourse.bass as bass
    import concourse.mybir as mybir
    import concourse.tile as tile
    from concourse import bacc

    f32 = mybir.dt.float32
    nc = bacc.Bacc("TRN2", target_bir_lowering=False, debug=False,
                   enable_asserts=False, num_devices=NCORES)
    repsT = nc.dram_tensor("repsT", [D, B], f32, kind="ExternalInput")
    embT = nc.dram_tensor("embT", [D, VS], f32, kind="ExternalInput")
    out = nc.dram_tensor("out", [B, VS], f32, kind="ExternalOutput")

    with tile.TileContext(nc) as tc:
        with (
            tc.tile_pool(name="const", bufs=1) as cpool,
            tc.tile_pool(name="eb", bufs=3) as ebp,
            tc.tile_pool(name="ob", bufs=3) as obp,
            tc.tile_pool(name="ps", bufs=8, space="PSUM") as psp,
        ):
            rt = cpool.tile([D, B], f32)
            nc.sync.dma_start(out=rt[:], in_=repsT[:, :])
            for i in range(VS // EB_COLS):
                eb = ebp.tile([D, EB_COLS], f32)
                nc.sync.dma_start(
                    out=eb[:], in_=embT[:, i * EB_COLS:(i + 1) * EB_COLS])
                for half in range(2):
                    ob = obp.tile([128, EB_COLS], f32)
                    for j in range(EB_COLS // CHUNK):
                        ps = psp.tile([128, CHUNK], f32)
                        nc.tensor.matmul(
                            ps[:],
                            rt[:, half * 128:(half + 1) * 128],
                            eb[:, j * CHUNK:(j + 1) * CHUNK],
                            start=True, stop=True)
                        nc.vector.tensor_copy(
                            out=ob[:, j * CHUNK:(j + 1) * CHUNK], in_=ps[:])
                    nc.sync.dma_start(
                        out=out[half * 128:(half + 1) * 128,
                                i * EB_COLS:(i + 1) * EB_COLS],
                        in_=ob[:])
    nc.compile()
    return nc


def _get_nc():
    global _NC
    if _NC is None:
        _NC = _build_nc()
    return _NC


LAST_EXEC_NS = None
LAST_RESULTS = None


def kernel(*, trace=False, **inputs):
    global LAST_EXEC_NS
    from concourse.bass_utils import run_bass_kernel_spmd

    reps = _host_reps(**inputs)                       # [B, D] fp32
    repsT = np.ascontiguousarray(reps.T)              # [D, B]
    emb = np.asarray(inputs["emb"], np.float32)
    embT = np.zeros((D, VP), np.float32)
    embT[:, :VTOT] = emb.T

    nc = _get_nc()
    in_maps = [
        {"repsT": repsT,
         "embT": np.ascontiguousarray(embT[:, c * VS:(c + 1) * VS])}
        for c in range(NCORES)
    ]
    if trace:
        try:
            res = run_bass_kernel_spmd(nc, in_maps,
                                       core_ids=list(range(NCORES)),
                                       trace=True)
        except (ImportError, ModuleNotFoundError):
            res = run_bass_kernel_spmd(nc, in_maps,
                                       core_ids=list(range(NCORES)))
    else:
        res = run_bass_kernel_spmd(nc, in_maps, core_ids=list(range(NCORES)))
    LAST_EXEC_NS = res.exec_time_ns
    logits = np.concatenate([r["out"] for r in res.results], axis=1)[:, :VTOT]
    return logits


# revision 5
# speedup vs baseline: 1.0557x; 1.0557x over previous
"""GCE-GNN session-rec forward for Trainium2.

Phase 1 (host, numpy): per-session graph construction + tiny GRU-style GNN
  (B=256 sessions, L=50, D=128 — ~0.5 GFLOP of irregular gather/scatter math).
Phase 2 (device, bass/tile, 8 NeuronCores): logits = reps @ emb.T
  vocab-sharded: each core reads a [128, VS] slice of emb.T and writes a
  [256, VS] slice of the output. This is the memory-bound bulk of the op
  (256 MB emb read + 512 MB logits write across the 8 cores).
"""

import numpy as np

V = 500000
L = 50
D = 128
B = 256
VTOT = V + 1

NCORES = 8
CHUNK = 512            # matmul moving-operand width (fp32 max)
EB_COLS = 2048         # emb.T columns per DMA (1 MiB per transfer)
VS = 31 * EB_COLS      # 63488 vocab columns per core
VP = VS * NCORES       # 507904 padded vocab


# ---------------------------------------------------------------------------
# Phase 1: host-side session GNN (numpy, float64 accumulation)
# ---------------------------------------------------------------------------

def _sigmoid(x):
    return 1.0 / (1.0 + np.exp(-x))


def _host_reps(seq, emb, W_in, W_out, Wz, bz, Uz, Wr, br, Ur, Wh, bh, Uh,
               Wg, bg, Wgate, bgate, Wproj, bproj):
    f = np.float64
    seq = np.asarray(seq)
    Bc, Lc = seq.shape
    BIG = emb.shape[0]  # sentinel > any valid item id

    valid = seq > 0
    lengths = valid.sum(1)

    # torch.unique(return_inverse) emulation, padded to L nodes
    sv = np.sort(np.where(valid, seq, BIG), axis=1)
    vs = sv < BIG
    is_new = vs & np.concatenate(
        [np.ones((Bc, 1), bool), sv[:, 1:] != sv[:, :-1]], axis=1)
    rank = np.cumsum(is_new, axis=1) - 1
    n_nodes = is_new.sum(1)
    buf = np.zeros((Bc, Lc + 1), sv.dtype)
    idx = np.where(is_new, rank, Lc)
    np.put_along_axis(buf, idx, sv, axis=1)
    uniq = buf[:, :Lc]
    usearch = np.where(np.arange(Lc)[None, :] < n_nodes[:, None], uniq, BIG)
    inv = np.empty((Bc, Lc), np.int64)
    for b in range(Bc):
        inv[b] = np.searchsorted(usearch[b], seq[b])
    inv = np.clip(inv, 0, Lc - 1)

    # local adjacency (binary), row-normalized
    pair_ok = valid[:, :-1] & valid[:, 1:]
    srcn = np.where(pair_ok, inv[:, :-1], 0)
    dstn = np.where(pair_ok, inv[:, 1:], 0)
    val = pair_ok.astype(f)
    multi = (n_nodes > 1).astype(f)[:, None, None]
    bidx = np.broadcast_to(np.arange(Bc)[:, None], srcn.shape)
    A_in = np.zeros((Bc, Lc, Lc), f)
    A_out = np.zeros((Bc, Lc, Lc), f)
    np.maximum.at(A_in, (bidx, dstn, srcn), val)
    np.maximum.at(A_out, (bidx, srcn, dstn), val)
    A_in *= multi
    A_out *= multi
    A_in /= (A_in.sum(2, keepdims=True) + 1e-8)
    A_out /= (A_out.sum(2, keepdims=True) + 1e-8)

    h = emb.astype(f)[uniq]  # [B, L, D]

    W_in, W_out, Wz, Uz, Wr, Ur, Wh, Uh, Wg, Wgate, Wproj = (
        a.astype(f) for a in (W_in, W_out, Wz, Uz, Wr, Ur, Wh, Uh, Wg, Wgate, Wproj))
    bz, br, bh, bg, bgate, bproj = (
        a.astype(f) for a in (bz, br, bh, bg, bgate, bproj))

    # local GRU-style GNN, one step
    m = A_in @ (h @ W_in) + A_out @ (h @ W_out)
    z = _sigmoid(m @ Wz + bz + h @ Uz)
    r = _sigmoid(m @ Wr + br + h @ Ur)
    ht = np.tanh(m @ Wh + bh + (r * h) @ Uh)
    h_local = (1.0 - z) * h + z * ht

    # global episode GNN, one step
    nvmask = (np.arange(Lc)[None, :] < n_nodes[:, None]).astype(f)
    Ag = nvmask[:, :, None] * nvmask[:, None, :] * \
        (1.0 - np.eye(Lc, dtype=f))[None]
    Ag /= (Ag.sum(2, keepdims=True) + 1e-8)
    h_global = np.where((n_nodes > 1)[:, None, None], Ag @ (h @ Wg + bg), h)

    # gather back to sequence, gate, attention pooling
    hl = np.take_along_axis(h_local, inv[:, :, None], axis=1)
    hg = np.take_along_axis(h_global, inv[:, :, None], axis=1)
    gate = _sigmoid(np.concatenate([hl, hg], axis=-1) @ Wgate + bgate)
    h_seq = gate * hl + (1.0 - gate) * hg
    last_idx = np.clip(lengths - 1, 0, Lc - 1)
    last_h = h_seq[np.arange(Bc), last_idx]
    att = np.where(valid, np.einsum('bld,bd->bl', h_seq, last_h), -1e9)
    att = att - att.max(1, keepdims=True)
    e = np.exp(att)
    alpha = e / e.sum(1, keepdims=True)
    s_g = np.einsum('bl,bld->bd', alpha, h_seq)
    reps = np.concatenate([s_g, last_h], axis=-1) @ Wproj + bproj
    return reps.astype(np.float32)  # [B, D]


# ---------------------------------------------------------------------------
# Phase 2: device kernel (built once, cached)
# ---------------------------------------------------------------------------

_NC = None


def _build_nc():
    import concourse.bass as bass
    import concourse.mybir as mybir
    import concourse.tile as tile
    from concourse import bacc

    f32 = mybir.dt.float32
    nc = bacc.Bacc("TRN2", target_bir_lowering=False, debug=False,
                   enable_asserts=False, num_devices=NCORES)
    repsT = nc.dram_tensor("repsT", [D, B], f32, kind="ExternalInput")
    embT = nc.dram_tensor("embT", [D, VS], f32, kind="ExternalInput")
    out = nc.dram_tensor("out", [B, VS], f32, kind="ExternalOutput")

    with tile.TileContext(nc) as tc:
        with (
            tc.tile_pool(name="const", bufs=1) as cpool,
            tc.tile_pool(name="eb", bufs=3) as ebp,
            tc.tile_pool(name="ob", bufs=3) as obp,
            tc.tile_pool(name="ps", bufs=8, space="PSUM") as psp,
        ):
            f32r = mybir.dt.float32r
            rt = cpool.tile([D, B], f32r)
            nc.sync.dma_start(out=rt[:], in_=repsT[:, :].bitcast(f32r))
            for i in range(VS // EB_COLS):
                eb = ebp.tile([D, EB_COLS], f32r)
                nc.sync.dma_start(
                    out=eb[:],
                    in_=embT[:, i * EB_COLS:(i + 1) * EB_COLS].bitcast(f32r))
                for half in range(2):
                    ob = obp.tile([128, EB_COLS], f32)
                    for j in range(EB_COLS // CHUNK):
                        ps = psp.tile([128, CHUNK], f32)
                        # float32r streams fp32 through the PE at full rate
                        # (1 cycle/row for moving dim >= 256) vs 4 for float32
                        nc.tensor.matmul(
                            ps[:],
                            rt[:, half * 128:(half + 1) * 128],
                            eb[:, j * CHUNK:(j + 1) * CHUNK],
                            start=True, stop=True)
                        nc.vector.tensor_copy(
                            out=ob[:, j * CHUNK:(j + 1) * CHUNK], in_=ps[:])
                    nc.sync.dma_start(
                        out=out[half * 128:(half + 1) * 128,
                                i * EB_COLS:(i + 1) * EB_COLS],
                        in_=ob[:])
    nc.compile()
    return nc


def _get_nc():
    global _NC
    if _NC is None:
        _NC = _build_nc()
    return _NC


LAST_EXEC_NS = None
LAST_RESULTS = None


def kernel(*, trace=False, **inputs):
    global LAST_EXEC_NS
    from concourse.bass_utils import run_bass_kernel_spmd

    reps = _host_reps(**inputs)                       # [B, D] fp32
    repsT = np.ascontiguousarray(reps.T)              # [D, B]
    emb = np.asarray(inputs["emb"], np.float32)
    embT = np.zeros((D, VP), np.float32)
    embT[:, :VTOT] = emb.T

    nc = _get_nc()
    in_maps = [
        {"repsT": repsT,
         "embT": np.ascontiguousarray(embT[:, c * VS:(c + 1) * VS])}
        for c in range(NCORES)
    ]
    if trace:
        try:
            res = run_bass_kernel_spmd(nc, in_maps,
                                       core_ids=list(range(NCORES)),
                                       trace=True)
        except (ImportError, ModuleNotFoundError):
            res = run_bass_kernel_spmd(nc, in_maps,
                                       core_ids=list(range(NCORES)))
    else:
        res = run_bass_kernel_spmd(nc, in_maps, core_ids=list(range(NCORES)))
    LAST_EXEC_NS = res.exec_time_ns
    logits = np.concatenate([r["out"] for r in res.results], axis=1)[:, :VTOT]
    return logits


# revision 6
# speedup vs baseline: 1.1470x; 1.0865x over previous
"""GCE-GNN session-rec forward for Trainium2.

Phase 1 (host, numpy): per-session graph construction + tiny GRU-style GNN
  (B=256 sessions, L=50, D=128 — ~0.5 GFLOP of irregular gather/scatter math).
Phase 2 (device, bass/tile, 8 NeuronCores): logits = reps @ emb.T
  vocab-sharded: each core reads a [128, VS] slice of emb.T and writes a
  [256, VS] slice of the output. This is the memory-bound bulk of the op
  (256 MB emb read + 512 MB logits write across the 8 cores).
"""

import numpy as np

V = 500000
L = 50
D = 128
B = 256
VTOT = V + 1

NCORES = 8
CHUNK = 512            # matmul moving-operand width (fp32 max)
EB_COLS = 2048         # emb.T columns per DMA (1 MiB per transfer)
VS = 31 * EB_COLS      # 63488 vocab columns per core
VP = VS * NCORES       # 507904 padded vocab


# ---------------------------------------------------------------------------
# Phase 1: host-side session GNN (numpy, float64 accumulation)
# ---------------------------------------------------------------------------

def _sigmoid(x):
    return 1.0 / (1.0 + np.exp(-x))


def _host_reps(seq, emb, W_in, W_out, Wz, bz, Uz, Wr, br, Ur, Wh, bh, Uh,
               Wg, bg, Wgate, bgate, Wproj, bproj):
    f = np.float64
    seq = np.asarray(seq)
    Bc, Lc = seq.shape
    BIG = emb.shape[0]  # sentinel > any valid item id

    valid = seq > 0
    lengths = valid.sum(1)

    # torch.unique(return_inverse) emulation, padded to L nodes
    sv = np.sort(np.where(valid, seq, BIG), axis=1)
    vs = sv < BIG
    is_new = vs & np.concatenate(
        [np.ones((Bc, 1), bool), sv[:, 1:] != sv[:, :-1]], axis=1)
    rank = np.cumsum(is_new, axis=1) - 1
    n_nodes = is_new.sum(1)
    buf = np.zeros((Bc, Lc + 1), sv.dtype)
    idx = np.where(is_new, rank, Lc)
    np.put_along_axis(buf, idx, sv, axis=1)
    uniq = buf[:, :Lc]
    usearch = np.where(np.arange(Lc)[None, :] < n_nodes[:, None], uniq, BIG)
    inv = np.empty((Bc, Lc), np.int64)
    for b in range(Bc):
        inv[b] = np.searchsorted(usearch[b], seq[b])
    inv = np.clip(inv, 0, Lc - 1)

    # local adjacency (binary), row-normalized
    pair_ok = valid[:, :-1] & valid[:, 1:]
    srcn = np.where(pair_ok, inv[:, :-1], 0)
    dstn = np.where(pair_ok, inv[:, 1:], 0)
    val = pair_ok.astype(f)
    multi = (n_nodes > 1).astype(f)[:, None, None]
    bidx = np.broadcast_to(np.arange(Bc)[:, None], srcn.shape)
    A_in = np.zeros((Bc, Lc, Lc), f)
    A_out = np.zeros((Bc, Lc, Lc), f)
    np.maximum.at(A_in, (bidx, dstn, srcn), val)
    np.maximum.at(A_out, (bidx, srcn, dstn), val)
    A_in *= multi
    A_out *= multi
    A_in /= (A_in.sum(2, keepdims=True) + 1e-8)
    A_out /= (A_out.sum(2, keepdims=True) + 1e-8)

    h = emb.astype(f)[uniq]  # [B, L, D]

    W_in, W_out, Wz, Uz, Wr, Ur, Wh, Uh, Wg, Wgate, Wproj = (
        a.astype(f) for a in (W_in, W_out, Wz, Uz, Wr, Ur, Wh, Uh, Wg, Wgate, Wproj))
    bz, br, bh, bg, bgate, bproj = (
        a.astype(f) for a in (bz, br, bh, bg, bgate, bproj))

    # local GRU-style GNN, one step
    m = A_in @ (h @ W_in) + A_out @ (h @ W_out)
    z = _sigmoid(m @ Wz + bz + h @ Uz)
    r = _sigmoid(m @ Wr + br + h @ Ur)
    ht = np.tanh(m @ Wh + bh + (r * h) @ Uh)
    h_local = (1.0 - z) * h + z * ht

    # global episode GNN, one step
    nvmask = (np.arange(Lc)[None, :] < n_nodes[:, None]).astype(f)
    Ag = nvmask[:, :, None] * nvmask[:, None, :] * \
        (1.0 - np.eye(Lc, dtype=f))[None]
    Ag /= (Ag.sum(2, keepdims=True) + 1e-8)
    h_global = np.where((n_nodes > 1)[:, None, None], Ag @ (h @ Wg + bg), h)

    # gather back to sequence, gate, attention pooling
    hl = np.take_along_axis(h_local, inv[:, :, None], axis=1)
    hg = np.take_along_axis(h_global, inv[:, :, None], axis=1)
    gate = _sigmoid(np.concatenate([hl, hg], axis=-1) @ Wgate + bgate)
    h_seq = gate * hl + (1.0 - gate) * hg
    last_idx = np.clip(lengths - 1, 0, Lc - 1)
    last_h = h_seq[np.arange(Bc), last_idx]
    att = np.where(valid, np.einsum('bld,bd->bl', h_seq, last_h), -1e9)
    att = att - att.max(1, keepdims=True)
    e = np.exp(att)
    alpha = e / e.sum(1, keepdims=True)
    s_g = np.einsum('bl,bld->bd', alpha, h_seq)
    reps = np.concatenate([s_g, last_h], axis=-1) @ Wproj + bproj
    return reps.astype(np.float32)  # [B, D]


# ---------------------------------------------------------------------------
# Phase 2: device kernel (built once, cached)
# ---------------------------------------------------------------------------

_NC = None


def _build_nc():
    import concourse.bass as bass
    import concourse.mybir as mybir
    import concourse.tile as tile
    from concourse import bacc

    f32 = mybir.dt.float32
    nc = bacc.Bacc("TRN2", target_bir_lowering=False, debug=False,
                   enable_asserts=False, num_devices=NCORES)
    repsT = nc.dram_tensor("repsT", [D, B], f32, kind="ExternalInput")
    embT = nc.dram_tensor("embT", [D, VS], f32, kind="ExternalInput")
    out = nc.dram_tensor("out", [B, VS], f32, kind="ExternalOutput")

    with tile.TileContext(nc) as tc:
        with (
            tc.tile_pool(name="const", bufs=1) as cpool,
            tc.tile_pool(name="eb", bufs=5) as ebp,
            tc.tile_pool(name="ob", bufs=6) as obp,
            tc.tile_pool(name="ps", bufs=8, space="PSUM") as psp,
        ):
            f32r = mybir.dt.float32r
            rt = cpool.tile([D, B], f32r)
            nc.sync.dma_start(out=rt[:], in_=repsT[:, :].bitcast(f32r))
            for i in range(VS // EB_COLS):
                eb = ebp.tile([D, EB_COLS], f32r)
                nc.sync.dma_start(
                    out=eb[:],
                    in_=embT[:, i * EB_COLS:(i + 1) * EB_COLS].bitcast(f32r))
                for half in range(2):
                    ob = obp.tile([128, EB_COLS], f32)
                    for j in range(EB_COLS // CHUNK):
                        ps = psp.tile([128, CHUNK], f32)
                        # float32r streams fp32 through the PE at full rate
                        # (1 cycle/row for moving dim >= 256) vs 4 for float32
                        nc.tensor.matmul(
                            ps[:],
                            rt[:, half * 128:(half + 1) * 128],
                            eb[:, j * CHUNK:(j + 1) * CHUNK],
                            start=True, stop=True)
                        nc.vector.tensor_copy(
                            out=ob[:, j * CHUNK:(j + 1) * CHUNK], in_=ps[:])
                    nc.sync.dma_start(
                        out=out[half * 128:(half + 1) * 128,
                                i * EB_COLS:(i + 1) * EB_COLS],
                        in_=ob[:])
    nc.compile()
    return nc


def _get_nc():
    global _NC
    if _NC is None:
        _NC = _build_nc()
    return _NC


LAST_EXEC_NS = None
LAST_RESULTS = None


def kernel(*, trace=False, **inputs):
    global LAST_EXEC_NS
    from concourse.bass_utils import run_bass_kernel_spmd

    reps = _host_reps(**inputs)                       # [B, D] fp32
    repsT = np.ascontiguousarray(reps.T)              # [D, B]
    emb = np.asarray(inputs["emb"], np.float32)
    embT = np.zeros((D, VP), np.float32)
    embT[:, :VTOT] = emb.T

    nc = _get_nc()
    in_maps = [
        {"repsT": repsT,
         "embT": np.ascontiguousarray(embT[:, c * VS:(c + 1) * VS])}
        for c in range(NCORES)
    ]
    if trace:
        try:
            res = run_bass_kernel_spmd(nc, in_maps,
                                       core_ids=list(range(NCORES)),
                                       trace=True)
        except (ImportError, ModuleNotFoundError):
            res = run_bass_kernel_spmd(nc, in_maps,
                                       core_ids=list(range(NCORES)))
    else:
        res = run_bass_kernel_spmd(nc, in_maps, core_ids=list(range(NCORES)))
    LAST_EXEC_NS = res.exec_time_ns
    logits = np.concatenate([r["out"] for r in res.results], axis=1)[:, :VTOT]
    return logits
